# revision 11
# baseline (speedup 1.0000x reference)
"""Multi-head attention (B=2, S=2048, E=1024, H=16, d=64) on 8 TRN2 cores.

Sharding: data-parallel over batch (2 groups of 4 cores), tensor-parallel over
heads (4 heads per core). The output projection is row-parallel; the partial
sums of the 4 cores in a batch group are reduced on the host during the
gather, together with the folded-out v-bias/out-bias correction term.

Per-core device program (all fp32):
  1. QKV projection from x^T (E-major):
       q^T, k^T per head-pair packed [128, 2048] (partitions 0-63 = head 2p,
       64-127 = head 2p+1), with the q/k biases added on PSUM evacuation.
       v S-major [128, 65] tiles per (head, k-chunk) with a ones column
       appended (yields the softmax denominator for free in the P.V matmul).
  2. scores^T = k^T.T @ q^T per head (K=64 row-tiled matmuls), exp via
     ScalarE with scale=1/sqrt(d) (inputs are small: no max subtraction
     needed for fp32 softmax here).
  3. attn = sum_kc exp^T[kc].T @ v_aug[kc] accumulated in PSUM; column 64 is
     the denominator; normalize with a per-partition reciprocal multiply.
  4. PE-transpose attn to E'-major, then out = attn @ W_o_slice.
"""

import sys

if "/opt/trn_rl_repo" not in sys.path:
    sys.path.insert(0, "/opt/trn_rl_repo")

import ml_dtypes
import numpy as np

import concourse.bacc as bacc
import concourse.mybir as mybir
import concourse.tile as tile
from concourse.bass_utils import run_bass_kernel_spmd
F32 = mybir.dt.float32
BF16 = mybir.dt.bfloat16

B = 2
S = 2048
E = 1024
H = 16
D = 64
N_CORES = 8
HPC = H // (N_CORES // B)  # heads per core = 4
EC = E // 128              # E chunks = 8
SC = S // 128              # S chunks = 16
EXP_BUFS = 20              # expT tiles in flight (one S_q half = 16 k-chunks)

_CACHE: dict = {}


def _build():
    nc = bacc.Bacc("TRN2", target_bir_lowering=False, debug=False,
                   num_devices=N_CORES)

    xT = nc.dram_tensor("xT", [E, S], BF16, kind="ExternalInput")
    w_q = nc.dram_tensor("w_q", [2, E, 128], BF16, kind="ExternalInput")
    w_k = nc.dram_tensor("w_k", [2, E, 128], BF16, kind="ExternalInput")
    b_q = nc.dram_tensor("b_q", [2, 128], F32, kind="ExternalInput")
    b_k = nc.dram_tensor("b_k", [2, 128], F32, kind="ExternalInput")
    w_v = nc.dram_tensor("w_v", [E, HPC * D], BF16, kind="ExternalInput")
    w_o = nc.dram_tensor("w_o", [HPC, 64, E], BF16, kind="ExternalInput")
    out = nc.dram_tensor("out", [S, E], F32, kind="ExternalOutput")

    xT_r = xT[:].rearrange("(e p) s -> p e s", p=128)
    wq_r = w_q[:].rearrange("t (e p) m -> p t e m", p=128)
    wk_r = w_k[:].rearrange("t (e p) m -> p t e m", p=128)
    wv_r = w_v[:].rearrange("(e p) m -> p e m", p=128)

    with tile.TileContext(nc) as tc:
        with (
            tc.tile_pool(name="wlong", bufs=1) as wlong,     # w_o, identity, biases
            tc.tile_pool(name="qk", bufs=1) as qkp,          # q^T / k^T pair tiles
            tc.tile_pool(name="vp", bufs=1) as vp,           # v tiles
            tc.tile_pool(name="attn", bufs=1) as attnp,      # normalized attn tiles
        ):
            ones_sb = wlong.tile([128, 128], BF16, tag="ones", name="ones")
            nc.vector.memset(ones_sb, 1.0)

            wo_sb = [wlong.tile([64, E], BF16, tag=f"wo{j}", name=f"wo{j}")
                     for j in range(HPC)]
            for j in range(HPC):
                nc.sync.dma_start(out=wo_sb[j], in_=w_o[j])

            bq_sb = [wlong.tile([128, 1], F32, tag=f"bq{p}", name=f"bq{p}") for p in range(2)]
            bk_sb = [wlong.tile([128, 1], F32, tag=f"bk{p}", name=f"bk{p}") for p in range(2)]
            for p in range(2):
                nc.sync.dma_start(out=bq_sb[p], in_=b_q[p, :, None])
                nc.sync.dma_start(out=bk_sb[p], in_=b_k[p, :, None])

            qT = [qkp.tile([128, S], BF16, tag=f"qT{p}", name=f"qT{p}") for p in range(2)]
            kT = [qkp.tile([128, S], BF16, tag=f"kT{p}", name=f"kT{p}") for p in range(2)]
            # v_aug tiles: per k-chunk, 4 heads x (64 v cols + 1 ones col)
            v_sb = [vp.tile([128, HPC * (D + 1)], BF16, tag=f"v{kc}", name=f"v{kc}")
                    for kc in range(SC)]
            for kc in range(SC):
                for h in range(HPC):
                    nc.vector.memset(v_sb[kc][:, h * 65 + 64: h * 65 + 65], 1.0)

            # ---- Phase A/B: loads + QKV projection ----
            with (
                tc.tile_pool(name="xp", bufs=1) as xp,
                tc.tile_pool(name="wqk", bufs=1) as wqkp,
                tc.tile_pool(name="ps_qk", bufs=3, space="PSUM") as ps_qk,
                tc.tile_pool(name="ps_v", bufs=3, space="PSUM") as ps_v,
            ):
                xt = [xp.tile([128, S], BF16, tag=f"xt{e}", name=f"xt{e}") for e in range(EC)]
                for e in range(EC):
                    nc.sync.dma_start(out=xt[e], in_=xT_r[:, e, :])

                wq_sb = [wqkp.tile([128, EC, 128], BF16, tag=f"wq{p}", name=f"wq{p}")
                         for p in range(2)]
                wk_sb = [wqkp.tile([128, EC, 128], BF16, tag=f"wk{p}", name=f"wk{p}")
                         for p in range(2)]
                wv_sb = wqkp.tile([128, EC, HPC * D], BF16, tag="wv")
                for p in range(2):
                    nc.sync.dma_start(out=wq_sb[p], in_=wq_r[:, p])
                    nc.sync.dma_start(out=wk_sb[p], in_=wk_r[:, p])
                nc.sync.dma_start(out=wv_sb, in_=wv_r)

                # q^T / k^T: out [128 (2 heads' d), 512-chunk of S]
                for p in range(2):
                    for (w_sb, dst, bias) in ((wq_sb[p], qT[p], bq_sb[p]),
                                              (wk_sb[p], kT[p], bk_sb[p])):
                        for ncq in range(S // 512):
                            ps = ps_qk.tile([128, 512], F32)
                            for e in range(EC):
                                nc.tensor.matmul(
                                    ps, lhsT=w_sb[:, e, :],
                                    rhs=xt[e][:, ncq * 512:(ncq + 1) * 512],
                                    start=(e == 0), stop=(e == EC - 1))
                            nc.vector.tensor_scalar_add(
                                dst[:, ncq * 512:(ncq + 1) * 512], ps, bias)

                # v: out [128-chunk of S, 256]; no bias (folded to host)
                for m in range(SC):
                    ps = ps_v.tile([128, HPC * D], F32)
                    for e in range(EC):
                        nc.tensor.matmul(
                            ps, lhsT=xt[e][:, m * 128:(m + 1) * 128],
                            rhs=wv_sb[:, e, :],
                            start=(e == 0), stop=(e == EC - 1))
                    for h in range(HPC):
                        nc.vector.tensor_copy(
                            v_sb[m][:, h * 65: h * 65 + 64],
                            ps[:, h * 64:(h + 1) * 64])

            # ---- Phase C: attention per head (PV-transposed) ----
            aTu = [attnp.tile([64, S], BF16, tag=f"aTu{h}", name=f"aTu{h}")
                   for h in range(HPC)]
            with (
                tc.tile_pool(name="expp", bufs=EXP_BUFS) as expp,
                tc.tile_pool(name="rp", bufs=2) as rp,
                tc.tile_pool(name="ps_s", bufs=2, space="PSUM") as ps_s,
                tc.tile_pool(name="ps_aT", bufs=1, space="PSUM") as ps_aT,
                tc.tile_pool(name="ps_bc", bufs=1, space="PSUM") as ps_bc,
            ):
                for h in range(HPC):
                    p, sub = h // 2, h % 2
                    lo, hi = sub * 64, (sub + 1) * 64
                    for half in range(2):
                        pat = ps_aT.tile([128, 1024], F32, name="pat")
                        for kc in range(SC):
                            etile = expp.tile([128, 1024], BF16, tag="expT",
                                              name="expT")
                            pss = ps_s.tile([128, 1024], F32, name="pss")
                            for j in range(2):
                                q0 = half * 1024 + j * 512
                                nc.tensor.matmul(
                                    pss[:, j * 512:(j + 1) * 512],
                                    lhsT=kT[p][lo:hi, kc * 128:(kc + 1) * 128],
                                    rhs=qT[p][lo:hi, q0:q0 + 512],
                                    start=True, stop=True)
                            nc.scalar.activation(
                                etile, pss,
                                mybir.ActivationFunctionType.Exp,
                                scale=float(1.0 / np.sqrt(D)))
                            for j in range(2):
                                nc.tensor.matmul(
                                    pat[0:65, j * 512:(j + 1) * 512],
                                    lhsT=v_sb[kc][:, h * 65:(h + 1) * 65],
                                    rhs=etile[:, j * 512:(j + 1) * 512],
                                    start=(kc == 0), stop=(kc == SC - 1))
                        # normalize: recip of denom row, broadcast, multiply
                        rrow = rp.tile([128, 1024], BF16, name="rrow")
                        with nc.allow_low_precision(reason="bf16 softmax recip"):
                            nc.vector.reciprocal(rrow[64:65, :],
                                                 pat[64:65, :])
                        pbc = ps_bc.tile([128, 1024], F32, name="pbc")
                        for j in range(2):
                            nc.tensor.matmul(
                                pbc[:, j * 512:(j + 1) * 512],
                                lhsT=ones_sb[64:65, :],
                                rhs=rrow[64:65, j * 512:(j + 1) * 512],
                                start=True, stop=True)
                        bc_sb = rp.tile([128, 1024], BF16, tag="bc",
                                        name="bc")
                        nc.vector.tensor_copy(bc_sb[0:64, :], pbc[0:64, :])
                        nc.vector.tensor_mul(
                            aTu[h][:, half * 1024:(half + 1) * 1024],
                            pat[0:64, :], bc_sb[0:64, :])

            # ---- Phase D: output projection (per-head K=64) ----
            with (
                tc.tile_pool(name="outp", bufs=3) as outp,
                tc.tile_pool(name="ps_o", bufs=4, space="PSUM") as ps_o,
            ):
                for m in range(SC):
                    o_sb = outp.tile([128, E], F32, tag="osb", name="osb")
                    for n2 in range(2):
                        pso = ps_o.tile([128, 512], F32, name="pso")
                        for h in range(HPC):
                            nc.tensor.matmul(
                                pso, lhsT=aTu[h][:, m * 128:(m + 1) * 128],
                                rhs=wo_sb[h][:, n2 * 512:(n2 + 1) * 512],
                                start=(h == 0), stop=(h == HPC - 1))
                        nc.vector.tensor_copy(
                            o_sb[:, n2 * 512:(n2 + 1) * 512], pso)
                    nc.sync.dma_start(
                        out=out[m * 128:(m + 1) * 128, :], in_=o_sb)

    nc.compile()
    return nc


def get_nc():
    if "nc" not in _CACHE:
        _CACHE["nc"] = _build()
    return _CACHE["nc"]


def make_in_maps(x, qkv_w, qkv_b, out_w):
    """Per-core input dicts for the SPMD kernel."""
    x = np.asarray(x, dtype=np.float32)
    qkv_w = np.asarray(qkv_w, dtype=np.float32)
    qkv_b = np.asarray(qkv_b, dtype=np.float32)
    out_w = np.asarray(out_w, dtype=np.float32)

    # reference layout: qkv.reshape(B, S, H, 3d) -> head h's q rows are
    # qkv_w[h*3d : h*3d+d], k rows +d, v rows +2d.
    def q_rows(h):
        return qkv_w[h * 3 * D: h * 3 * D + D]

    def k_rows(h):
        return qkv_w[h * 3 * D + D: h * 3 * D + 2 * D]

    def v_rows(h):
        return qkv_w[h * 3 * D + 2 * D: h * 3 * D + 3 * D]

    xT = [np.ascontiguousarray(x[b].T) for b in range(B)]
    in_maps = []
    for c in range(N_CORES):
        b = c // (N_CORES // B)
        hs = (c % (N_CORES // B)) * HPC
        w_q = np.stack([
            np.ascontiguousarray(np.concatenate(
                [q_rows(hs + 2 * p), q_rows(hs + 2 * p + 1)], axis=0).T)
            for p in range(2)])
        w_k = np.stack([
            np.ascontiguousarray(np.concatenate(
                [k_rows(hs + 2 * p), k_rows(hs + 2 * p + 1)], axis=0).T)
            for p in range(2)])
        b_qv = np.stack([
            np.concatenate([qkv_b[(hs + 2 * p + i) * 3 * D:
                                  (hs + 2 * p + i) * 3 * D + D]
                            for i in range(2)])
            for p in range(2)])
        b_kv = np.stack([
            np.concatenate([qkv_b[(hs + 2 * p + i) * 3 * D + D:
                                  (hs + 2 * p + i) * 3 * D + 2 * D]
                            for i in range(2)])
            for p in range(2)])
        w_v = np.ascontiguousarray(np.concatenate(
            [v_rows(hs + i) for i in range(HPC)], axis=0).T)
        w_o = np.ascontiguousarray(
            out_w[:, hs * D:(hs + HPC) * D].T).reshape(HPC, 64, E)
        bf = ml_dtypes.bfloat16
        in_maps.append({
            "xT": xT[b].astype(bf), "w_q": w_q.astype(bf),
            "w_k": w_k.astype(bf), "b_q": b_qv, "b_k": b_kv,
            "w_v": w_v.astype(bf), "w_o": w_o.astype(bf),
        })
    return in_maps


def unshard(results, qkv_b, out_w, out_b):
    """Sum row-parallel partials per batch group + bias correction."""
    qkv_b = np.asarray(qkv_b, dtype=np.float32)
    out_w = np.asarray(out_w, dtype=np.float32)
    out_b = np.asarray(out_b, dtype=np.float32)
    # v-bias passes through softmax unchanged; fold through out proj here.
    bv_full = np.concatenate(
        [qkv_b[h * 3 * D + 2 * D: h * 3 * D + 3 * D] for h in range(H)])
    corr = out_w @ bv_full + out_b
    gpb = N_CORES // B
    out = np.empty((B, S, E), dtype=np.float32)
    for b in range(B):
        acc = results[b * gpb]["out"].astype(np.float32).copy()
        for c in range(b * gpb + 1, (b + 1) * gpb):
            acc += results[c]["out"]
        out[b] = acc + corr[None, :]
    return out


def kernel(x, qkv_w, qkv_b, out_w, out_b):
    nc = get_nc()
    in_maps = make_in_maps(x, qkv_w, qkv_b, out_w)
    res = run_bass_kernel_spmd(nc, in_maps, list(range(N_CORES)))
    return unshard(res.results, qkv_b, out_w, out_b)
